# revision 85
# baseline (speedup 1.0000x reference)
"""Trainium2 Bass kernel for nn_LiquidGenerator.

score = sum over (i, image j) pairs of (CUTOFF - dist)^2 where dist < CUTOFF,
with dist over the [N, 27N] supercell distance matrix.

Strategy
--------
Host (O(N) prep):
  * generate P (rotation+translation of molecule-local coords, float64)
  * shift symmetry d(i,(k,j)) == d(j,(-k,i)):
        score = sum_full(central) + 2 * sum_full(13 half-shifts)
    so only 14 of the 27 images are computed.
  * per-shift j-banding: a pair can only contribute if every coordinate gap
    |P_i,c - (P_j,c + v_c)| < CUTOFF, so for each shift only columns j with
    P_j,c + v_c inside [min_i P_i,c - 3, max_i P_i,c + 3] (all c) are kept.
    That leaves ~300 of 13*1024 shifted columns -> ~4x less device work,
    exactly (the dropped pairs provably contribute zero).
  * distances via a 5-feature inner product:
      d^2 + BIAS = [Px,Py,Pz,|P|^2,1] . [-2Sx,-2Sy,-2Sz, 1, |S|^2+BIAS]
    (coordinates centered at the cell midpoint for fp32 accuracy; BIAS keeps
    the PE-accumulated value strictly positive so ACT Sqrt never sees a
    negative input).

Device (8 NeuronCores, j-sharded): each core takes its 128 central columns
plus every-8th banded shifted column (padded with dummy d^2>9 columns to a
fixed WB=512). i runs over all 1024 rows as 8 chunks of 128. Chunks are
processed 4 at a time: four fp32 matmuls (K=5) packed into the four 32-row
groups of the PE array run concurrently, each writing its own PSUM bank of a
[128, 2048] tile. Then:
  ScalarE : Sqrt activation [128,2048] PSUM->SBUF
  VectorE : v = min(s,3)-3 (tensor_scalar, 2x mode); v^2-sum via
            scalar_tensor_tensor with accum_out, central and shifted parts
            reduced with 3D APs (weights applied on host)
  GPSIMD  : stages each chunk's central s-block for the exact host-side
            self-pair correction
Host: fp64 sum of partials, weight 2 on shifted blocks, exact removal of the
device-computed self-pair terms (recomputed from the read-back s values) and
addition of the analytic self-pair contribution N*(3-sqrt(EPS))^2.
"""

import numpy as np

CUTOFF = 3.0
EPS = 1e-16
BIAS = 2e-4
BAND_MARGIN = 1e-3  # slack on the banding interval vs fp32 rounding

NCORES = 8
N = 1024             # 128 molecules x 8 atoms
JC = N // NCORES     # 128 central columns per core
NCHUNK = 8           # i-chunks of 128
G = 4                # concurrent PE row groups (chunks per super-chunk)
SC = NCHUNK // G     # super-chunks
WB = 512             # banded columns per core: 128 central + <=384 shifted
LC = SC * 128        # lhs columns per row group (one 128-block per super-chunk)
FEAT_W = LC + WB + 64

_cache: dict = {}


# ----------------------------------------------------------------- host math
def _rotation_matrices(rot):
    a, b, g = rot[:, 0], rot[:, 1], rot[:, 2]
    ca, sa = np.cos(a), np.sin(a)
    cb, sb = np.cos(b), np.sin(b)
    cg, sg = np.cos(g), np.sin(g)
    m = rot.shape[0]
    rx = np.zeros((m, 3, 3)); ry = np.zeros((m, 3, 3)); rz = np.zeros((m, 3, 3))
    rx[:, 0, 0] = 1;  rx[:, 1, 1] = ca; rx[:, 1, 2] = -sa; rx[:, 2, 1] = sa; rx[:, 2, 2] = ca
    ry[:, 0, 0] = cb; ry[:, 0, 2] = -sb; ry[:, 1, 1] = 1;  ry[:, 2, 0] = sb; ry[:, 2, 2] = cb
    rz[:, 0, 0] = cg; rz[:, 0, 1] = -sg; rz[:, 1, 0] = sg; rz[:, 1, 1] = cg; rz[:, 2, 2] = 1
    return np.einsum("mij,mjk,mkl->mil", rx, ry, rz)


def _generate(positions, translation, rotation, cell):
    R = _rotation_matrices(rotation.astype(np.float64))
    trans = np.remainder(translation.astype(np.float64), 1.0) @ cell.astype(np.float64)
    gen = np.einsum("mai,mij->maj", positions.astype(np.float64), R) + trans[:, None, :]
    return gen.reshape(-1, 3)


def _features(S, c, bias):
    """rhs feature columns for image positions S (pairs with lhs features)."""
    Sc = (S - c).astype(np.float32)
    return np.stack([
        -2.0 * Sc[:, 0], -2.0 * Sc[:, 1], -2.0 * Sc[:, 2],
        np.ones(S.shape[0], np.float32),
        (Sc.astype(np.float64) ** 2).sum(1).astype(np.float32) + np.float32(bias),
    ]).astype(np.float32)


# ------------------------------------------------------------- bass program
def _build_program(reps: int = 1, loop_n: int = 0):
    key = ("nc", reps, loop_n)
    if key in _cache:
        return _cache[key]
    from contextlib import ExitStack, nullcontext
    import concourse.tile as tile
    from concourse import bacc, mybir

    f32 = mybir.dt.float32
    # Bacc (not raw Bass): its compile() runs the wait-legalization passes
    # (move_matmul_waits_to_ldweights / generate_event_semaphores, one sync
    # wait per instruction on this walrus) plus ACT table-load insertion.
    nc = bacc.Bacc("TRN2", target_bir_lowering=False, debug=False,
                   num_devices=NCORES)
    # lhs and rhs features bundled in one [128, *] tensor: a single DMA -> a
    # single sync wait on the first matmul, full-partition DMA bandwidth, and
    # replicas of the 5 feature rows at partition offsets 32t so row-group-
    # packed matmuls can run concurrently. 64 zero tail columns double as the
    # bf16-zero operand pool for the toucher matmuls.
    feat_d = nc.dram_tensor("feat", [128, FEAT_W], f32, kind="ExternalInput")
    acc_d = nc.dram_tensor("acc", [128, 4 * SC], f32, kind="ExternalOutput")
    sdiag_d = nc.dram_tensor("sdiag", [128, NCHUNK * JC], f32,
                             kind="ExternalOutput")

    with tile.TileContext(nc) as tc, ExitStack() as ctx:
        const = ctx.enter_context(tc.tile_pool(name="const", bufs=1))
        psum = ctx.enter_context(tc.tile_pool(name="psum", bufs=2, space="PSUM"))
        spool = ctx.enter_context(tc.tile_pool(name="s", bufs=4))
        scrap = ctx.enter_context(tc.tile_pool(name="scrap", bufs=3))

        ft = const.tile([128, FEAT_W], f32)
        nc.sync.dma_start(ft[:], feat_d[:])
        at = const.tile([128, 4 * SC], f32)
        sall = const.tile([128, NCHUNK * JC], f32)

        # bf16-zero views of the zero-padded feat tail for "toucher" matmuls
        bw = ft[0:1, LC + WB:LC + WB + 64].bitcast(mybir.dt.bfloat16)  # [1,128]
        bx = bw[:, 0:1]

        loop_cm = tc.For_i(0, loop_n, 1) if loop_n else nullcontext()
        with loop_cm:
            for u in range(SC * reps):
                s = u % SC
                ps = psum.tile([128, G * WB], f32)
                for t in range(G):
                    # group t handles i-chunk s*G+t in PE row group 32t; the
                    # four fp32 matmuls execute concurrently, one PSUM bank
                    # each
                    nc.tensor.matmul(
                        ps[:, t * WB:(t + 1) * WB],
                        ft[32 * t:32 * t + 5, s * 128:(s + 1) * 128],
                        ft[32 * t:32 * t + 5, LC:LC + WB],
                        start=True, stop=True,
                        tile_position=(32 * t, 0),
                    )
                st = spool.tile([128, G * WB], f32)
                vt = scrap.tile([128, G * WB], f32)
                # sqrt per PSUM bank (= per row-group matmul) so each starts
                # as soon as its own matmul drains, and the DVE pipelines
                # right behind the ACT
                for h in range(G):
                    nc.scalar.activation(st[:, h * WB:(h + 1) * WB],
                                         ps[:, h * WB:(h + 1) * WB],
                                         mybir.ActivationFunctionType.Sqrt)
                    # v = min(s, 3) - 3  ->  v^2 == relu(3-s)^2
                    nc.vector.tensor_scalar(
                        vt[:, h * WB:(h + 1) * WB], st[:, h * WB:(h + 1) * WB],
                        CUTOFF, CUTOFF,
                        mybir.AluOpType.min, mybir.AluOpType.subtract,
                    )
                # Toucher: after ACT has read the PSUM tile, a 1-column bf16
                # matmul re-takes PSUM ownership on the PE with a single ACT
                # wait, so the next super-chunk's fp32 matmuls (which can
                # encode at most one wait) only ever see a same-engine dep.
                nc.tensor.matmul(ps[:, 0:1], bw, bx, start=True, stop=True)
                # stash central s blocks for the exact self-pair correction
                for t in range(G):
                    ic = s * G + t
                    nc.gpsimd.tensor_copy(sall[:, ic * JC:(ic + 1) * JC],
                                          st[:, t * WB:t * WB + JC])
                sq = scrap.tile([128, G * WB], f32, tag="sqout")
                v3 = vt[:].rearrange("p (g n) -> p g n", g=G)
                q3 = sq[:].rearrange("p (g n) -> p g n", g=G)
                # square+reduce in two halves (2 groups each) so the first can
                # run while ts still works on the second half's groups;
                # central (weight 1) and banded shifted (weight 2) separated
                for h in range(2):
                    gs = slice(2 * h, 2 * h + 2)
                    col = 4 * s + 2 * h
                    nc.vector.scalar_tensor_tensor(
                        q3[:, gs, 0:JC], v3[:, gs, 0:JC], 1.0, v3[:, gs, 0:JC],
                        mybir.AluOpType.mult, mybir.AluOpType.mult,
                        accum_out=at[:, col:col + 1],
                    )
                    nc.vector.scalar_tensor_tensor(
                        q3[:, gs, JC:WB], v3[:, gs, JC:WB], 1.0,
                        v3[:, gs, JC:WB],
                        mybir.AluOpType.mult, mybir.AluOpType.mult,
                        accum_out=at[:, col + 1:col + 2],
                    )

        nc.sync.dma_start(acc_d[:], at[:])
        nc.sync.dma_start(sdiag_d[:], sall[:])

    # Bacc.finalize runs compile(): wait legalization, ACT table loads,
    # register allocation.
    nc.finalize()
    _cache[key] = nc
    return nc


# --------------------------------------------------------------- input prep
def _prepare_inputs(positions, translation, rotation, cell):
    cell64 = cell.astype(np.float64)
    P = _generate(positions, translation, rotation, cell64)      # [N,3] float64
    n = P.shape[0]
    assert n == N, f"kernel hardcodes N={N}, got {n}"

    shifts = np.array([-1.0, 0.0, 1.0])
    offs = np.stack(np.meshgrid(shifts, shifts, shifts, indexing="ij")).reshape(3, -1).T
    vecs = offs @ cell64                                          # [27,3]
    assert np.all(offs[13] == 0.0)
    half = list(range(13))                                        # pairs with 26-k

    c = 0.5 * cell64.sum(axis=0)
    Pc = (P - c).astype(np.float32)
    lhsT = np.stack([
        Pc[:, 0], Pc[:, 1], Pc[:, 2],
        (Pc.astype(np.float64) ** 2).sum(1).astype(np.float32),
        np.ones(n, np.float32),
    ]).astype(np.float32)                                         # [5, N]

    # per-shift j-banding: keep only columns that can possibly be within
    # CUTOFF of ANY i in ANY coordinate (exact necessary condition)
    lo = P.min(axis=0) - (CUTOFF + BAND_MARGIN)
    hi = P.max(axis=0) + (CUTOFF + BAND_MARGIN)
    cand = []
    for k in half:
        S = P + vecs[k]
        mask = np.all((S > lo) & (S < hi), axis=1)
        cand.append(np.nonzero(mask)[0])

    dummy = np.array([0.0, 0.0, 0.0, 1.0, 100.0], np.float32).reshape(5, 1)

    in_maps = []
    for core in range(NCORES):
        cols = [_features(P[core * JC:(core + 1) * JC], c, BIAS)]  # central
        nsh = 0
        for k in half:
            idx = cand[k][core::NCORES]
            if idx.size:
                cols.append(_features(P[idx] + vecs[k], c, BIAS))
                nsh += idx.size
        assert JC + nsh <= WB, (
            f"banded columns {JC + nsh} exceed WB={WB}; raise WB"
        )
        cols.append(np.repeat(dummy, WB - JC - nsh, axis=1))
        rhs = np.concatenate(cols, axis=1)                        # [5, WB]

        feat = np.zeros((128, FEAT_W), np.float32)
        for t in range(G):
            # lhs block for row group t: one 128-column block per super-chunk
            for s in range(SC):
                ic = s * G + t
                feat[32 * t:32 * t + 5, s * 128:(s + 1) * 128] = \
                    lhsT[:, ic * 128:(ic + 1) * 128]
            feat[32 * t:32 * t + 5, LC:LC + WB] = rhs
        in_maps.append({"feat": np.ascontiguousarray(feat)})
    return in_maps


# ------------------------------------------------------------------- runner
def _get_runner(reps: int = 1, loop_n: int = 0):
    """Jit the bass program once; reuse the compiled executable per call.

    (bass2jax.run_bass_via_pjrt rebuilds its jit closure every call, paying
    retrace + executable reload each time.)
    """
    key = ("runner", reps, loop_n)
    if key in _cache:
        return _cache[key]
    import jax
    from jax.sharding import Mesh, PartitionSpec
    from jax.experimental.shard_map import shard_map
    from concourse import bass2jax, mybir

    nc = _build_program(reps=reps, loop_n=loop_n)
    bass2jax.install_neuronx_cc_hook()

    partition_name = (
        nc.partition_id_tensor.name if nc.partition_id_tensor else None
    )
    in_names, out_names, out_avals, zero_outs = [], [], [], []
    for alloc in nc.m.functions[0].allocations:
        if not isinstance(alloc, mybir.MemoryLocationSet):
            continue
        name = alloc.memorylocations[0].name
        if alloc.kind == "ExternalInput":
            if name != partition_name:
                in_names.append(name)
        elif alloc.kind == "ExternalOutput":
            out_names.append(name)
            shape = tuple(alloc.tensor_shape)
            dtype = mybir.dt.np(alloc.dtype)
            out_avals.append(jax.core.ShapedArray(shape, dtype))
            zero_outs.append(np.zeros(shape, dtype))
    n_params = len(in_names)
    all_in_names = in_names + out_names
    if partition_name is not None:
        all_in_names = all_in_names + [partition_name]

    def _body(*args):
        operands = list(args)
        if partition_name is not None:
            operands.append(bass2jax.partition_id_tensor())
        outs = bass2jax._bass_exec_p.bind(
            *operands,
            out_avals=tuple(out_avals),
            in_names=tuple(all_in_names),
            out_names=tuple(out_names),
            lowering_input_output_aliases=(),
            sim_require_finite=True,
            sim_require_nnan=True,
            nc=nc,
        )
        return tuple(outs)

    devices = jax.devices()[:NCORES]
    mesh = Mesh(np.asarray(devices), ("core",))
    n_outs = len(out_names)
    sharded = jax.jit(
        shard_map(
            _body, mesh=mesh,
            in_specs=(PartitionSpec("core"),) * (n_params + n_outs),
            out_specs=(PartitionSpec("core"),) * n_outs,
            check_rep=False,
        ),
        keep_unused=True,
    )
    concat_zeros = [
        np.zeros((NCORES * z.shape[0], *z.shape[1:]), z.dtype) for z in zero_outs
    ]

    def run(in_maps):
        concat_in = [
            np.concatenate([in_maps[cc][name] for cc in range(NCORES)], axis=0)
            for name in in_names
        ]
        out_arrs = sharded(*concat_in, *concat_zeros)
        return [
            {
                name: np.asarray(out_arrs[i]).reshape(NCORES, *out_avals[i].shape)[cc]
                for i, name in enumerate(out_names)
            }
            for cc in range(NCORES)
        ]

    _cache[key] = run
    return run


def kernel(positions, translation, rotation, cell, _reps=1, _loop_n=0):
    run = _get_runner(reps=_reps, loop_n=_loop_n)
    in_maps = _prepare_inputs(
        np.asarray(positions), np.asarray(translation),
        np.asarray(rotation), np.asarray(cell),
    )
    results = run(in_maps)
    total = 0.0
    for core, r in enumerate(results):
        acc = r["acc"].astype(np.float64)
        total += acc[:, 0::2].sum() + 2.0 * acc[:, 1::2].sum()
        # exact removal of the device-computed self-pair terms: the diagonal
        # of this core's central block, recomputed from the exact s values
        s_pp = r["sdiag"][:, core * JC:(core + 1) * JC].diagonal()
        v = (np.minimum(s_pp, np.float32(CUTOFF)) - np.float32(CUTOFF)).astype(np.float32)
        total -= (v.astype(np.float64) ** 2).sum()
    total += N * (CUTOFF - np.sqrt(np.float32(EPS))) ** 2  # exact self pairs
    return np.float32(total)


# revision 95
# speedup vs baseline: 5.3247x; 5.3247x over previous
"""Trainium2 Bass kernel for nn_LiquidGenerator.

score = sum over (i, image j) pairs of (CUTOFF - dist)^2 where dist < CUTOFF,
with dist over the [N, 27N] supercell distance matrix.

Strategy
--------
Host (O(N) prep):
  * generate P (rotation+translation of molecule-local coords, float64)
  * shift symmetry d(i,(k,j)) == d(j,(-k,i)):
        score = sum_full(central) + 2 * sum_full(13 half-shifts)
    so only 14 of the 27 images are computed.
  * per-shift j-banding: a pair can only contribute if every coordinate gap
    |P_i,c - (P_j,c + v_c)| < CUTOFF, so for each shift only columns j with
    P_j,c + v_c inside [min_i P_i,c - 3, max_i P_i,c + 3] (all c) are kept.
    That leaves ~300 of 13*1024 shifted columns -> ~4x less device work,
    exactly (the dropped pairs provably contribute zero).
  * distances via a 5-feature inner product:
      d^2 + BIAS = [Px,Py,Pz,|P|^2,1] . [-2Sx,-2Sy,-2Sz, 1, |S|^2+BIAS]
    (coordinates centered at the cell midpoint for fp32 accuracy; BIAS keeps
    the PE-accumulated value strictly positive so ACT Sqrt never sees a
    negative input).

Device (8 NeuronCores, j-sharded): each core takes its 128 central columns
plus every-8th banded shifted column (padded with dummy d^2>9 columns to a
fixed WB=512). i runs over all 1024 rows as 8 chunks of 128. Chunks are
processed 4 at a time: four fp32 matmuls (K=5) packed into the four 32-row
groups of the PE array run concurrently, each writing its own PSUM bank of a
[128, 2048] tile. Then:
  ScalarE : Sqrt activation [128,2048] PSUM->SBUF
  VectorE : v = min(s,3)-3 (tensor_scalar, 2x mode); v^2-sum via
            scalar_tensor_tensor with accum_out, central and shifted parts
            reduced with 3D APs (weights applied on host)
  GPSIMD  : stages each chunk's central s-block for the exact host-side
            self-pair correction
Host: fp64 sum of partials, weight 2 on shifted blocks, exact removal of the
device-computed self-pair terms (recomputed from the read-back s values) and
addition of the analytic self-pair contribution N*(3-sqrt(EPS))^2.
"""

import numpy as np

CUTOFF = 3.0
EPS = 1e-16
BIAS = 2e-4
BAND_MARGIN = 1e-3  # slack on the banding interval vs fp32 rounding

NCORES = 8
N = 1024             # 128 molecules x 8 atoms
JC = N // NCORES     # 128 central columns per core
NCHUNK = 8           # i-chunks of 128
G = 4                # concurrent PE row groups (chunks per super-chunk)
SC = NCHUNK // G     # super-chunks
WB = 512             # banded columns per core: 128 central + <=384 shifted
LC = SC * 128        # lhs columns per row group (one 128-block per super-chunk)
FEAT_W = LC + WB + 64

_cache: dict = {}


# ----------------------------------------------------------------- host math
def _rotation_matrices(rot):
    a, b, g = rot[:, 0], rot[:, 1], rot[:, 2]
    ca, sa = np.cos(a), np.sin(a)
    cb, sb = np.cos(b), np.sin(b)
    cg, sg = np.cos(g), np.sin(g)
    m = rot.shape[0]
    rx = np.zeros((m, 3, 3)); ry = np.zeros((m, 3, 3)); rz = np.zeros((m, 3, 3))
    rx[:, 0, 0] = 1;  rx[:, 1, 1] = ca; rx[:, 1, 2] = -sa; rx[:, 2, 1] = sa; rx[:, 2, 2] = ca
    ry[:, 0, 0] = cb; ry[:, 0, 2] = -sb; ry[:, 1, 1] = 1;  ry[:, 2, 0] = sb; ry[:, 2, 2] = cb
    rz[:, 0, 0] = cg; rz[:, 0, 1] = -sg; rz[:, 1, 0] = sg; rz[:, 1, 1] = cg; rz[:, 2, 2] = 1
    return np.einsum("mij,mjk,mkl->mil", rx, ry, rz)


def _generate(positions, translation, rotation, cell):
    R = _rotation_matrices(rotation.astype(np.float64))
    trans = np.remainder(translation.astype(np.float64), 1.0) @ cell.astype(np.float64)
    gen = np.einsum("mai,mij->maj", positions.astype(np.float64), R) + trans[:, None, :]
    return gen.reshape(-1, 3)


def _features(S, c, bias):
    """rhs feature columns for image positions S (pairs with lhs features)."""
    Sc = (S - c).astype(np.float32)
    return np.stack([
        -2.0 * Sc[:, 0], -2.0 * Sc[:, 1], -2.0 * Sc[:, 2],
        np.ones(S.shape[0], np.float32),
        (Sc.astype(np.float64) ** 2).sum(1).astype(np.float32) + np.float32(bias),
    ]).astype(np.float32)


# ------------------------------------------------------------- bass program
def _build_program(reps: int = 1, loop_n: int = 0, rw: int = WB):
    # rw: the number of real (non-dummy) columns per group, <= WB; engines
    # only touch [0, rw) of each group while groups stay WB-strided so each
    # matmul still owns a whole PSUM bank.
    key = ("nc", reps, loop_n, rw)
    if key in _cache:
        return _cache[key]
    from contextlib import ExitStack, nullcontext
    import concourse.tile as tile
    from concourse import bacc, mybir

    f32 = mybir.dt.float32
    # Bacc (not raw Bass): its compile() runs the wait-legalization passes
    # (move_matmul_waits_to_ldweights / generate_event_semaphores, one sync
    # wait per instruction on this walrus) plus ACT table-load insertion.
    nc = bacc.Bacc("TRN2", target_bir_lowering=False, debug=False,
                   num_devices=NCORES)
    # lhs and rhs features bundled in one [128, *] tensor: a single DMA -> a
    # single sync wait on the first matmul, full-partition DMA bandwidth, and
    # replicas of the 5 feature rows at partition offsets 32t so row-group-
    # packed matmuls can run concurrently. 64 zero tail columns double as the
    # bf16-zero operand pool for the toucher matmuls.
    feat_d = nc.dram_tensor("feat", [128, FEAT_W], f32, kind="ExternalInput")
    acc_d = nc.dram_tensor("acc", [128, 4 * SC], f32, kind="ExternalOutput")
    sdiag_d = nc.dram_tensor("sdiag", [128, NCHUNK * JC], f32,
                             kind="ExternalOutput")

    with tile.TileContext(nc) as tc, ExitStack() as ctx:
        const = ctx.enter_context(tc.tile_pool(name="const", bufs=1))
        psum = ctx.enter_context(tc.tile_pool(name="psum", bufs=2, space="PSUM"))
        spool = ctx.enter_context(tc.tile_pool(name="s", bufs=4))
        scrap = ctx.enter_context(tc.tile_pool(name="scrap", bufs=3))

        ft = const.tile([128, FEAT_W], f32)
        nc.sync.dma_start(ft[:], feat_d[:])
        at = const.tile([128, 4 * SC], f32)
        sall = const.tile([128, NCHUNK * JC], f32)

        # bf16-zero views of the zero-padded feat tail for "toucher" matmuls
        bw = ft[0:1, LC + WB:LC + WB + 64].bitcast(mybir.dt.bfloat16)  # [1,128]
        bx = bw[:, 0:1]

        loop_cm = tc.For_i(0, loop_n, 1) if loop_n else nullcontext()
        with loop_cm:
            for u in range(SC * reps):
                s = u % SC
                ps = psum.tile([128, G * WB], f32)
                for t in range(G):
                    # group t handles i-chunk s*G+t in PE row group 32t; the
                    # four fp32 matmuls execute concurrently, one PSUM bank
                    # each
                    nc.tensor.matmul(
                        ps[:, t * WB:t * WB + rw],
                        ft[32 * t:32 * t + 5, s * 128:(s + 1) * 128],
                        ft[32 * t:32 * t + 5, LC:LC + rw],
                        start=True, stop=True,
                        tile_position=(32 * t, 0),
                    )
                st = spool.tile([128, G * WB], f32)
                vt = scrap.tile([128, G * WB], f32)
                # sqrt per PSUM bank (= per row-group matmul) so each starts
                # as soon as its own matmul drains, and the DVE pipelines
                # right behind the ACT
                for h in range(G):
                    nc.scalar.activation(st[:, h * WB:h * WB + rw],
                                         ps[:, h * WB:h * WB + rw],
                                         mybir.ActivationFunctionType.Sqrt)
                    # v = min(s, 3) - 3  ->  v^2 == relu(3-s)^2
                    nc.vector.tensor_scalar(
                        vt[:, h * WB:h * WB + rw], st[:, h * WB:h * WB + rw],
                        CUTOFF, CUTOFF,
                        mybir.AluOpType.min, mybir.AluOpType.subtract,
                    )
                # Toucher: after ACT has read the PSUM tile, a 1-column bf16
                # matmul re-takes PSUM ownership on the PE with a single ACT
                # wait, so the next super-chunk's fp32 matmuls (which can
                # encode at most one wait) only ever see a same-engine dep.
                nc.tensor.matmul(ps[:, 0:1], bw, bx, start=True, stop=True)
                # stash central s blocks for the exact self-pair correction
                for t in range(G):
                    ic = s * G + t
                    nc.gpsimd.tensor_copy(sall[:, ic * JC:(ic + 1) * JC],
                                          st[:, t * WB:t * WB + JC])
                sq = scrap.tile([128, G * WB], f32, tag="sqout")
                v3 = vt[:].rearrange("p (g n) -> p g n", g=G)
                q3 = sq[:].rearrange("p (g n) -> p g n", g=G)
                # square+reduce in two halves (2 groups each) so the first can
                # run while ts still works on the second half's groups;
                # central (weight 1) and banded shifted (weight 2) separated
                for h in range(2):
                    gs = slice(2 * h, 2 * h + 2)
                    col = 4 * s + 2 * h
                    nc.vector.scalar_tensor_tensor(
                        q3[:, gs, 0:JC], v3[:, gs, 0:JC], 1.0, v3[:, gs, 0:JC],
                        mybir.AluOpType.mult, mybir.AluOpType.mult,
                        accum_out=at[:, col:col + 1],
                    )
                    nc.vector.scalar_tensor_tensor(
                        q3[:, gs, JC:rw], v3[:, gs, JC:rw], 1.0,
                        v3[:, gs, JC:rw],
                        mybir.AluOpType.mult, mybir.AluOpType.mult,
                        accum_out=at[:, col + 1:col + 2],
                    )

        nc.sync.dma_start(acc_d[:], at[:])
        nc.sync.dma_start(sdiag_d[:], sall[:])

    # Bacc.finalize runs compile(): wait legalization, ACT table loads,
    # register allocation.
    nc.finalize()
    _cache[key] = nc
    return nc


# --------------------------------------------------------------- input prep
def _prepare_inputs(positions, translation, rotation, cell):
    cell64 = cell.astype(np.float64)
    P = _generate(positions, translation, rotation, cell64)      # [N,3] float64
    n = P.shape[0]
    assert n == N, f"kernel hardcodes N={N}, got {n}"

    shifts = np.array([-1.0, 0.0, 1.0])
    offs = np.stack(np.meshgrid(shifts, shifts, shifts, indexing="ij")).reshape(3, -1).T
    vecs = offs @ cell64                                          # [27,3]
    assert np.all(offs[13] == 0.0)
    half = list(range(13))                                        # pairs with 26-k

    c = 0.5 * cell64.sum(axis=0)
    Pc = (P - c).astype(np.float32)
    lhsT = np.stack([
        Pc[:, 0], Pc[:, 1], Pc[:, 2],
        (Pc.astype(np.float64) ** 2).sum(1).astype(np.float32),
        np.ones(n, np.float32),
    ]).astype(np.float32)                                         # [5, N]

    # per-shift j-banding: keep only columns that can possibly be within
    # CUTOFF of ANY i in ANY coordinate (exact necessary condition)
    lo = P.min(axis=0) - (CUTOFF + BAND_MARGIN)
    hi = P.max(axis=0) + (CUTOFF + BAND_MARGIN)
    cand = []
    for k in half:
        S = P + vecs[k]
        mask = np.all((S > lo) & (S < hi), axis=1)
        cand.append(np.nonzero(mask)[0])

    dummy = np.array([0.0, 0.0, 0.0, 1.0, 100.0], np.float32).reshape(5, 1)

    # real columns per group: central + worst-core shifted count, rounded up
    # to a multiple of 8 (keeps even innermost dims for DVE packed modes)
    nsh_max = max(
        sum(cand[k][core::NCORES].size for k in half) for core in range(NCORES)
    )
    rw = min(WB, (JC + nsh_max + 7) // 8 * 8)

    in_maps = []
    for core in range(NCORES):
        cols = [_features(P[core * JC:(core + 1) * JC], c, BIAS)]  # central
        nsh = 0
        for k in half:
            idx = cand[k][core::NCORES]
            if idx.size:
                cols.append(_features(P[idx] + vecs[k], c, BIAS))
                nsh += idx.size
        assert JC + nsh <= rw <= WB, (
            f"banded columns {JC + nsh} exceed WB={WB}; raise WB"
        )
        cols.append(np.repeat(dummy, WB - JC - nsh, axis=1))
        rhs = np.concatenate(cols, axis=1)                        # [5, WB]

        feat = np.zeros((128, FEAT_W), np.float32)
        for t in range(G):
            # lhs block for row group t: one 128-column block per super-chunk
            for s in range(SC):
                ic = s * G + t
                feat[32 * t:32 * t + 5, s * 128:(s + 1) * 128] = \
                    lhsT[:, ic * 128:(ic + 1) * 128]
            feat[32 * t:32 * t + 5, LC:LC + WB] = rhs
        in_maps.append({"feat": np.ascontiguousarray(feat)})
    return in_maps, rw


# ------------------------------------------------------------------- runner
def _get_runner(reps: int = 1, loop_n: int = 0, rw: int = WB):
    """Jit the bass program once; reuse the compiled executable per call.

    (bass2jax.run_bass_via_pjrt rebuilds its jit closure every call, paying
    retrace + executable reload each time.)
    """
    key = ("runner", reps, loop_n, rw)
    if key in _cache:
        return _cache[key]
    import jax
    from jax.sharding import Mesh, PartitionSpec
    from jax.experimental.shard_map import shard_map
    from concourse import bass2jax, mybir

    nc = _build_program(reps=reps, loop_n=loop_n, rw=rw)
    bass2jax.install_neuronx_cc_hook()

    partition_name = (
        nc.partition_id_tensor.name if nc.partition_id_tensor else None
    )
    in_names, out_names, out_avals, zero_outs = [], [], [], []
    for alloc in nc.m.functions[0].allocations:
        if not isinstance(alloc, mybir.MemoryLocationSet):
            continue
        name = alloc.memorylocations[0].name
        if alloc.kind == "ExternalInput":
            if name != partition_name:
                in_names.append(name)
        elif alloc.kind == "ExternalOutput":
            out_names.append(name)
            shape = tuple(alloc.tensor_shape)
            dtype = mybir.dt.np(alloc.dtype)
            out_avals.append(jax.core.ShapedArray(shape, dtype))
            zero_outs.append(np.zeros(shape, dtype))
    n_params = len(in_names)
    all_in_names = in_names + out_names
    if partition_name is not None:
        all_in_names = all_in_names + [partition_name]

    def _body(*args):
        operands = list(args)
        if partition_name is not None:
            operands.append(bass2jax.partition_id_tensor())
        outs = bass2jax._bass_exec_p.bind(
            *operands,
            out_avals=tuple(out_avals),
            in_names=tuple(all_in_names),
            out_names=tuple(out_names),
            lowering_input_output_aliases=(),
            sim_require_finite=True,
            sim_require_nnan=True,
            nc=nc,
        )
        return tuple(outs)

    devices = jax.devices()[:NCORES]
    mesh = Mesh(np.asarray(devices), ("core",))
    n_outs = len(out_names)
    sharded = jax.jit(
        shard_map(
            _body, mesh=mesh,
            in_specs=(PartitionSpec("core"),) * (n_params + n_outs),
            out_specs=(PartitionSpec("core"),) * n_outs,
            check_rep=False,
        ),
        keep_unused=True,
    )
    concat_zeros = [
        np.zeros((NCORES * z.shape[0], *z.shape[1:]), z.dtype) for z in zero_outs
    ]

    def run(in_maps):
        concat_in = [
            np.concatenate([in_maps[cc][name] for cc in range(NCORES)], axis=0)
            for name in in_names
        ]
        out_arrs = sharded(*concat_in, *concat_zeros)
        return [
            {
                name: np.asarray(out_arrs[i]).reshape(NCORES, *out_avals[i].shape)[cc]
                for i, name in enumerate(out_names)
            }
            for cc in range(NCORES)
        ]

    _cache[key] = run
    return run


def kernel(positions, translation, rotation, cell, _reps=1, _loop_n=0):
    in_maps, rw = _prepare_inputs(
        np.asarray(positions), np.asarray(translation),
        np.asarray(rotation), np.asarray(cell),
    )
    run = _get_runner(reps=_reps, loop_n=_loop_n, rw=rw)
    results = run(in_maps)
    total = 0.0
    for core, r in enumerate(results):
        acc = r["acc"].astype(np.float64)
        total += acc[:, 0::2].sum() + 2.0 * acc[:, 1::2].sum()
        # exact removal of the device-computed self-pair terms: the diagonal
        # of this core's central block, recomputed from the exact s values
        s_pp = r["sdiag"][:, core * JC:(core + 1) * JC].diagonal()
        v = (np.minimum(s_pp, np.float32(CUTOFF)) - np.float32(CUTOFF)).astype(np.float32)
        total -= (v.astype(np.float64) ** 2).sum()
    total += N * (CUTOFF - np.sqrt(np.float32(EPS))) ** 2  # exact self pairs
    return np.float32(total)
